# revision 10
# baseline (speedup 1.0000x reference)
"""Trainium2 Bass kernel v3 for nn_PhysicsInformedKinematicLoss.

Decoupled DMA/compute granularity: 8 uniform 1024-col load groups, but the
last loads' DMAs are split into column pieces (SUBS_MAP) and compute runs
per-piece on views, so the post-DMA drain chain shrinks with the piece size
(subtile deps give per-piece readiness).

Engine assignment per block (stats lag one block):
  ACT : fa, fb, fc, dmg(relu+accum), kse, sq_{k-1}(Square+accum)
  Pool: ab, num (+ks on full blocks)
  DVE : d_{k-1}(+Sd), Std_{k-1}(+acc), [ks tail], recip, A2, scan_u, scan_p
Block-local ramp (1..sz); host restores global ramp via Std_k += off_k*Sd_k.
"""

import os

import numpy as np

from concourse import bacc, bass, tile
from concourse import mybir
from concourse.bass_utils import run_bass_kernel_spmd
from contextlib import ExitStack

F32 = mybir.dt.float32
Alu = mybir.AluOpType
Act = mybir.ActivationFunctionType

B = 512
S = 16384
NCORE = 8
RPC = B // NCORE          # 64
H = S // 2                # 8192
DT = 0.01
ALPHA = 0.01
ETA = 0.001

LOAD_W = 1024
NL = H // LOAD_W          # 8
SUBS_MAP = {}
TAIL_AB_DVE = False       # ab on DVE for sub-blocks (else Pool)
TAIL_KS_DVE = True        # ks on DVE for sub-blocks (else Pool)
KSE_DVE_TAIL = False      # kse via stt add/bypass on DVE for sub-blocks
USE_DIVIDE = False        # A2 = (num*DT^2) / kse in one stt (no recip)

CBLK = []
NB = 0
OFFS = []


def _plan():
    global CBLK, NB, OFFS
    CBLK = []
    for l in range(NL):
        subs = SUBS_MAP.get(l, [LOAD_W])
        o = 0
        for s in subs:
            CBLK.append((l, o, s))
            o += s
        assert o == LOAD_W
    NB = len(CBLK)
    OFFS = [l * LOAD_W + o for (l, o, s) in CBLK]


def _build_program():
    _plan()
    nds = 2 * NB
    nas = 2 * NB + 2
    nc = bacc.Bacc("TRN2", target_bir_lowering=False, debug=False,
                   num_devices=NCORE)
    a_ap = nc.dram_tensor("a_dirty", [RPC, S], F32, kind="ExternalInput").ap()
    st_ap = nc.dram_tensor("states", [RPC, S, 4], F32,
                           kind="ExternalInput").ap()
    pg_ap = nc.dram_tensor("p_gt", [RPC, S], F32, kind="ExternalInput").ap()
    dve_ap = nc.dram_tensor("dve_stats", [128, nds], F32,
                            kind="ExternalOutput").ap()
    act_ap = nc.dram_tensor("act_stats", [128, nas], F32,
                            kind="ExternalOutput").ap()

    with tile.TileContext(nc) as tc, ExitStack() as ctx:
        const_pool = ctx.enter_context(tc.tile_pool(name="const", bufs=1))
        ramp = const_pool.tile([128, LOAD_W], F32)
        nc.gpsimd.iota(ramp[:], pattern=[[1, LOAD_W]], base=1,
                       channel_multiplier=0,
                       allow_small_or_imprecise_dtypes=True)
        dve_st = const_pool.tile([128, nds], F32)
        act_st = const_pool.tile([128, nas], F32)

        a_pool = ctx.enter_context(tc.tile_pool(name="a_in", bufs=3))
        pg_pool = ctx.enter_context(tc.tile_pool(name="pg_in", bufs=3))
        st_pool = ctx.enter_context(tc.tile_pool(name="st_in", bufs=3))
        work = ctx.enter_context(tc.tile_pool(name="work", bufs=2))

        def dma_piece(t, ap2, o, s, c0, inner=None):
            lo, hi = c0 + o, c0 + o + s
            if inner is None:
                nc.sync.dma_start(t[0:64, o:o + s], ap2[:, lo:hi])
                nc.sync.dma_start(t[64:128, o:o + s], ap2[:, H + lo:H + hi])
            else:
                nc.sync.dma_start(t[0:64, o:o + s, :], ap2[:, lo:hi, :])
                nc.sync.dma_start(t[64:128, o:o + s, :],
                                  ap2[:, H + lo:H + hi, :])

        loaded = -1
        a_t = pg_t = st_t = None
        u_prev = p_prev = d_prev = None
        pg_prev = None
        prev_sz = 0
        for k, (l, o, sz) in enumerate(CBLK):
            if l > loaded:
                c0 = l * LOAD_W
                subs = SUBS_MAP.get(l, [LOAD_W])
                a_t = a_pool.tile([128, LOAD_W], F32, name="a_t")
                pg_t = pg_pool.tile([128, LOAD_W], F32, name="pg_t")
                st_t = st_pool.tile([128, LOAD_W, 4], F32, name="st_t")
                if len(subs) == 1:
                    dma_piece(a_t, a_ap, 0, LOAD_W, c0)
                    dma_piece(pg_t, pg_ap, 0, LOAD_W, c0)
                    dma_piece(st_t, st_ap, 0, LOAD_W, c0, inner=4)
                else:
                    so = 0
                    for ssz in subs:
                        dma_piece(st_t, st_ap, so, ssz, c0, inner=4)
                        dma_piece(a_t, a_ap, so, ssz, c0)
                        dma_piece(pg_t, pg_ap, so, ssz, c0)
                        so += ssz
                loaded = l

            Dv = st_t[:, o:o + sz, 0]
            Tv = st_t[:, o:o + sz, 1]
            sgv = st_t[:, o:o + sz, 2]
            crv = st_t[:, o:o + sz, 3]
            a_v = a_t[:, o:o + sz]
            pg_v = pg_t[:, o:o + sz]

            if k > 0:
                # DVE: d_{k-1} = p_{k-1} - pg_{k-1}  (+Sd accum)
                d_t = work.tile([128, prev_sz], F32)
                nc.vector.scalar_tensor_tensor(
                    d_t[:], p_prev[:], 0.0, pg_prev, Alu.bypass,
                    Alu.subtract, accum_out=dve_st[:, k - 1:k])
                # DVE: Stdloc_{k-1} = d*ramp (+accum)
                std_scr = work.tile([128, prev_sz], F32)
                nc.vector.scalar_tensor_tensor(
                    std_scr[:], d_t[:], 0.0, ramp[:, 0:prev_sz], Alu.bypass,
                    Alu.mult, accum_out=dve_st[:, NB + k - 1:NB + k])
                d_prev = d_t

            full = sz == LOAD_W
            t1 = work.tile([128, sz], F32)
            t2 = work.tile([128, sz], F32)
            t3 = work.tile([128, sz], F32)
            t4 = work.tile([128, sz], F32)
            t5 = work.tile([128, sz], F32)
            scr = work.tile([128, max(sz, prev_sz)], F32)

            # ACT: factors
            nc.scalar.activation(t1[:], Dv, Act.Copy, bias=1.0, scale=-1.0)
            nc.scalar.activation(t2[:], Tv, Act.Copy, bias=1.0, scale=ALPHA)
            nc.scalar.activation(t3[:], sgv, Act.Copy, bias=1.0, scale=-ETA)
            # ab = fa*fb ; num = a - creep
            if full or not TAIL_AB_DVE:
                nc.gpsimd.tensor_tensor(t5[:], t1[:], t2[:], Alu.mult)
            else:
                nc.vector.scalar_tensor_tensor(t5[:], t1[:], 0.0, t2[:],
                                               Alu.bypass, Alu.mult)
            nc.gpsimd.tensor_tensor(t4[:], a_v, crv, Alu.subtract)
            # ks = ab*fc
            if full or not TAIL_KS_DVE:
                nc.gpsimd.tensor_tensor(t1[:], t5[:], t3[:], Alu.mult)
            else:
                nc.vector.scalar_tensor_tensor(t1[:], t5[:], 0.0, t3[:],
                                               Alu.bypass, Alu.mult)
            # ACT: dmg = relu(-D) accum
            nc.scalar.activation(scr[:, 0:sz], Dv, Act.Relu, bias=0.0,
                                 scale=-1.0,
                                 accum_out=act_st[:, NB + k:NB + k + 1])
            # kse = ks + 1e-6
            if KSE_DVE_TAIL and not full:
                nc.vector.scalar_tensor_tensor(t5[:], t1[:], 1e-6, t1[:],
                                               Alu.add, Alu.bypass)
            else:
                nc.scalar.activation(t5[:], t1[:], Act.Copy, bias=1e-6,
                                     scale=1.0)
            if k > 0:
                # ACT: sq_{k-1} = d^2 accum -> Sd2
                nc.scalar.activation(
                    scr[:, 0:prev_sz], d_prev[:], Act.Square, bias=0.0,
                    scale=1.0, accum_out=act_st[:, k - 1:k])
            # DVE: A2 = (num*DT^2) / kse
            if USE_DIVIDE:
                nc.vector.scalar_tensor_tensor(t1[:], t4[:], DT * DT, t5[:],
                                               Alu.mult, Alu.divide)
            else:
                nc.vector.reciprocal_approx_fast(t2[:], t5[:])
                nc.vector.scalar_tensor_tensor(t1[:], t4[:], DT * DT, t2[:],
                                               Alu.mult, Alu.mult)
            # DVE: scans
            u_t = work.tile([128, sz], F32)
            p_t = work.tile([128, sz], F32)
            u_init = 0.0 if k == 0 else u_prev[:, prev_sz - 1:prev_sz]
            p_init = 0.0 if k == 0 else p_prev[:, prev_sz - 1:prev_sz]
            nc.vector.tensor_tensor_scan(u_t[:], t1[:], t1[:], u_init,
                                         Alu.add, Alu.bypass)
            nc.vector.tensor_tensor_scan(p_t[:], u_t[:], u_t[:], p_init,
                                         Alu.add, Alu.bypass)
            u_prev, p_prev = u_t, p_t
            pg_prev = pg_v
            prev_sz = sz

        # epilogue: stats for last block
        d_t = work.tile([128, prev_sz], F32)
        nc.vector.scalar_tensor_tensor(
            d_t[:], p_prev[:], 0.0, pg_prev, Alu.bypass, Alu.subtract,
            accum_out=dve_st[:, NB - 1:NB])
        std2 = work.tile([128, prev_sz], F32, name="std_scr")
        nc.vector.scalar_tensor_tensor(
            std2[:], d_t[:], 0.0, ramp[:, 0:prev_sz], Alu.bypass, Alu.mult,
            accum_out=dve_st[:, 2 * NB - 1:2 * NB])
        sq2 = work.tile([128, prev_sz], F32, name="scr")
        nc.scalar.activation(sq2[:], d_t[:], Act.Square, bias=0.0, scale=1.0,
                             accum_out=act_st[:, NB - 1:NB])
        nc.scalar.copy(act_st[:, 2 * NB:2 * NB + 1],
                       u_prev[:, prev_sz - 1:prev_sz])
        nc.scalar.copy(act_st[:, 2 * NB + 1:2 * NB + 2],
                       p_prev[:, prev_sz - 1:prev_sz])
        nc.sync.dma_start(dve_ap[:], dve_st[:])
        nc.sync.dma_start(act_ap[:], act_st[:])

    nc.compile()
    return nc


_NC_CACHE = None
LAST_EXEC_NS = None


def kernel(**inputs: np.ndarray) -> np.ndarray:
    global _NC_CACHE, LAST_EXEC_NS
    a = np.ascontiguousarray(inputs["a_dirty"], dtype=np.float32)
    st = np.ascontiguousarray(inputs["states"], dtype=np.float32)
    pg = np.ascontiguousarray(inputs["p_gt"], dtype=np.float32)
    v0 = np.asarray(inputs["v_0"], dtype=np.float64)
    p0 = np.asarray(inputs["p_0"], dtype=np.float64)

    if _NC_CACHE is None:
        _NC_CACHE = _build_program()
    nc = _NC_CACHE

    in_maps = []
    for c in range(NCORE):
        r0, r1 = c * RPC, (c + 1) * RPC
        in_maps.append({
            "a_dirty": a[r0:r1],
            "states": st[r0:r1],
            "p_gt": pg[r0:r1],
        })
    trace = os.environ.get("KERNEL_TRACE", "") == "1"
    res = run_bass_kernel_spmd(nc, in_maps, core_ids=list(range(NCORE)),
                               trace=trace)
    LAST_EXEC_NS = res.exec_time_ns

    pairs = [(np.asarray(res.results[c]["dve_stats"]),
              np.asarray(res.results[c]["act_stats"])) for c in range(NCORE)]
    return _finalize(pairs, v0, p0)


def _finalize(pairs, v0, p0):
    T1 = H * (H + 1) / 2.0
    T2 = H * (H + 1) * (2 * H + 1) / 6.0
    total_sq = 0.0
    total_dmg = 0.0
    offs = np.array(OFFS, dtype=np.float64)
    for c in range(NCORE):
        dve = np.asarray(pairs[c][0], dtype=np.float64)
        act = np.asarray(pairs[c][1], dtype=np.float64)
        Sd_j = dve[:, 0:NB]
        Sd = Sd_j.sum(axis=1)
        Std = (dve[:, NB:2 * NB] + offs[None, :] * Sd_j).sum(axis=1)
        Sd2 = act[:, 0:NB].sum(axis=1)
        total_dmg += act[:, NB:2 * NB].sum()
        u_last = act[:, 2 * NB]
        p_last = act[:, 2 * NB + 1]

        v0c = v0[c * RPC:(c + 1) * RPC]
        p0c = p0[c * RPC:(c + 1) * RPC]
        Cu = np.empty(128)
        Cp = np.empty(128)
        Cu[0:64] = DT * v0c
        Cp[0:64] = p0c
        Cu[64:128] = DT * v0c + u_last[0:64]
        Cp[64:128] = p0c + H * DT * v0c + p_last[0:64]

        sq = (Sd2 + Cp * Cp * H + Cu * Cu * T2 + 2.0 * Cp * Sd
              + 2.0 * Cu * Std + 2.0 * Cp * Cu * T1)
        total_sq += sq.sum()

    n = float(B) * float(S)
    loss = total_sq / n + 0.1 * total_dmg / n
    return np.array(loss, dtype=np.float32)
